# revision 33
# baseline (speedup 1.0000x reference)
"""Trainium2 Bass kernel for CustomBSplineLayer.

Computes out[b,o] = sum_{i,g} spline(x)[b,i,g] * coef[o,i,g] where
spline is an order-3 (cubic) B-spline basis on uniform knots applied to
tanh(x).

Math used here (validated against the reference recursion):
  u = 3.5*tanh(x) + 3.5           in (0, 7)
  basis_g(u) = M4(u - g)          cardinal cubic B-spline, g = 0..7
  6*M4(s) = relu(2-|s-2|)^3 - 4*relu(1-|s-2|)^3
Plane g=7 is identically zero because its support starts at u=7 ==
tanh upper bound, so only 7 of 8 planes contribute (K = 7*1024 = 7168).
The 1/6 and the 4 are folded host-side into coef (coef/6) and into the
rq scale (4^(1/3)).

Per-core layout (data-parallel over batch, 8 cores x 512 rows):
  - host pre-transposes x so tiles arrive as [i partitions, b cols]
  - the whole basis pipeline runs in fp16: DVE tensor ops get the 2x
    (tensor_tensor) / 4x (tensor_scalar) perf modes, and fp16 matmul
    streams at 1 col/cycle on the PE like fp32r but with half the
    coef DMA traffic.
  - engine balance per i-tile (cost model): ACT ~10.1us (tanh, two
    wide Relus, one wide Square), DVE ~10.1us (7 w-planes, int16
    abs-AND, 3 wide cubes + subtract), PE ~11.95us -> PE-bound.
  - basis planes [i, b] feed the PE as the stationary (lhsT) operand;
    coef (host-rearranged to [g, i, o], fp16, pre-divided by 6) is the
    moving operand; out accumulates in PSUM as [b, o] across all 56
    k-tiles, then is copied out once.
"""

import sys

sys.path.insert(0, "/opt/trn_rl_repo")

import numpy as np
from contextlib import ExitStack

import concourse.bass as bass
import concourse.tile as tile
from concourse import bacc, mybir
from concourse.bass_utils import run_bass_kernel_spmd

F32 = mybir.dt.float32
F16 = mybir.dt.float16
I16 = mybir.dt.int16
AF = mybir.ActivationFunctionType
OP = mybir.AluOpType

B, I, O = 4096, 1024, 1024
G = 7                    # active basis planes (plane 7 == 0)
NCORES = 8
BC = B // NCORES         # 512 batch rows per core
IT = I // 128            # 8 i-tiles
KT = IT * G              # 56 k-tiles of 128
WID = G * BC             # 3584: wide free-dim (7 planes x 512 b)

CQ = float(4.0 ** (1.0 / 3.0))   # folds the 4 into rq: rq^3 = 4*relu(1-a)^3

MM_DT = F16

LAST_RESULT = None  # BassKernelResults of the most recent run (for test.py)

_cache = {}


def _build_nc(repeats: int = 1):
    nc = bacc.Bacc("TRN2", target_bir_lowering=False, debug=False)
    xT = nc.dram_tensor("xT", [I, BC], F32, kind="ExternalInput").ap()
    coefT = nc.dram_tensor("coefT", [G, I, O], MM_DT, kind="ExternalInput").ap()
    # fp16 output (half the drain DMA); host converts back to f32
    y = nc.dram_tensor("y", [BC, O], F16, kind="ExternalOutput").ap()

    with tile.TileContext(nc) as tc, ExitStack() as ctx:
        xt_pool = ctx.enter_context(tc.tile_pool(name="xt", bufs=1))
        small = ctx.enter_context(tc.tile_pool(name="small", bufs=2))
        wide = ctx.enter_context(tc.tile_pool(name="wide", bufs=2))
        spl_pool = ctx.enter_context(tc.tile_pool(name="spl", bufs=2))
        rhs_pool = ctx.enter_context(tc.tile_pool(name="rhs", bufs=8))
        out_pool = ctx.enter_context(tc.tile_pool(name="ot", bufs=2))
        psum_pool = ctx.enter_context(
            tc.tile_pool(name="psum", bufs=1, space=bass.MemorySpace.PSUM)
        )

        consts = ctx.enter_context(tc.tile_pool(name="consts", bufs=1))
        bias_p = consts.tile([128, 1], F32, tag="bias_p", name="bias_p")
        nc.gpsimd.memset(bias_p[:], 2.0)
        bias_q = consts.tile([128, 1], F32, tag="bias_q", name="bias_q")
        nc.gpsimd.memset(bias_q[:], CQ)
        bias_abs = consts.tile([128, 2], F32, tag="bias_abs", name="bias_abs")
        for g in range(2):
            nc.gpsimd.memset(bias_abs[:, g : g + 1], float(1.5 - g))
        # all-zeros tile for PE warmup matmuls: accumulating 0*x is an
        # exact no-op, so these keep the tensor engine busy (and its DVFS
        # ramp warm) while the first basis tiles are still being computed
        zmm = consts.tile([128, 512], F16, tag="zmm", name="zmm")
        nc.vector.memset(zmm[:], 0.0)

        def emit_warmup(n, first=False):
            for i in range(n):
                nc.tensor.matmul(
                    psum[i % 4][(i // 4) % 2][:],
                    zmm[:, :128],
                    zmm[:],
                    start=(first and i < 8),
                    stop=False,
                )

        WARM_FILL = 38   # ~8.1us of zero-matmuls covering the basis fill
        WARM_T1 = 10     # ~2.1us covering the tile-0 -> tile-1 transition

        # 8 PSUM banks: [m-tile 0..3] x [o-half 0..1], each [128, 512] f32
        psum = [
            [
                psum_pool.tile(
                    [128, 512], F32, tag=f"ps{m}_{h}", name=f"ps{m}_{h}"
                )
                for h in range(2)
            ]
            for m in range(4)
        ]

        def emit_basis(rep, it, xt, bounds=((0, G),), act_abs_planes=(), bsplit=0):
            """tanh + the full fp16 basis pipeline for i-tile `it` (xt is
            the pre-DMA'd [128, BC] f32 input tile).

            Returns the [128, WID] fp16 spline tile (7 g-planes x 512 b).
            `bounds` is a tuple of (g_start, g_end) plane groups: fine
            groups let early tiles feed the PE sooner (deps are tracked
            per slice); one full-width group has the least op overhead.
            `bsplit` planes at the head are additionally emitted in
            batch-halves to shorten the startup latency chain.
            """
            t = small.tile([128, BC], F16, tag="t", name=f"t{rep}_{it}")
            if bsplit:
                nc.scalar.activation(t[:, : BC // 2], xt[:, : BC // 2], AF.Tanh)
                nc.scalar.activation(t[:, BC // 2 :], xt[:, BC // 2 :], AF.Tanh)
            else:
                nc.scalar.activation(t[:], xt[:], AF.Tanh)

            # w_g = 3.5*t + (1.5-g) per plane (DVE ts, 4x mode in fp16),
            # then int16 AND clears the sign bit: a = |w|
            aw = wide.tile([128, WID], F16, tag="a", name=f"aw{rep}_{it}")
            rp = wide.tile([128, WID], F16, tag="rp", name=f"rp{rep}_{it}")
            rq = wide.tile([128, WID], F16, tag="rq", name=f"rq{rep}_{it}")
            p2 = wide.tile([128, WID], F16, tag="p2", name=f"p2{rep}_{it}")
            q2 = wide.tile([128, WID], F16, tag="q2", name=f"q2{rep}_{it}")
            spl = spl_pool.tile([128, WID], F16, tag="spl", name=f"spl{rep}_{it}")
            for g0, g1 in bounds:
                if all(g in act_abs_planes for g in range(g0, g1)):
                    # latency path (startup): a = |3.5t + (1.5-g)| in ONE
                    # ACT op so the whole chain to q2 stays on ACT with no
                    # cross-engine semaphore hops
                    for g in range(g0, g1):
                        nc.scalar.activation(
                            aw[:, g * BC : (g + 1) * BC],
                            t[:],
                            AF.Abs,
                            bias=bias_abs[:, g : g + 1],
                            scale=3.5,
                        )
                else:
                    for g in range(g0, g1):
                        nc.vector.tensor_scalar(
                            aw[:, g * BC : (g + 1) * BC],
                            t[:],
                            3.5,
                            float(1.5 - g),
                            OP.mult,
                            OP.add,
                        )
                    s = slice(g0 * BC, g1 * BC)
                    awi = aw[:, s].bitcast(I16)
                    nc.vector.tensor_scalar(awi, awi, 0x7FFF, None, OP.bitwise_and)
                s = slice(g0 * BC, g1 * BC)
                # ACT: rp = relu(2 - a); rq = CQ*relu(1 - a); q2 = rq^2
                nc.scalar.activation(
                    rp[:, s], aw[:, s], AF.Relu, bias=bias_p[:], scale=-1.0
                )
                nc.scalar.activation(
                    rq[:, s], aw[:, s], AF.Relu, bias=bias_q[:], scale=-CQ
                )
                nc.scalar.activation(q2[:, s], rq[:, s], AF.Square)
                # DVE (2x fp16): p2 = rp^2, p3 = p2*rp, q3 = q2*rq (in-place)
                nc.vector.tensor_tensor(p2[:, s], rp[:, s], rp[:, s], OP.mult)
                nc.vector.tensor_tensor(p2[:, s], p2[:, s], rp[:, s], OP.mult)
                nc.vector.tensor_tensor(q2[:, s], q2[:, s], rq[:, s], OP.mult)
                # spl = p3 - q3, per plane so the PE's per-g lhsT dependency
                # is satisfied as early as possible (and emitted inside the
                # group so the scheduler doesn't defer it)
                for g in range(g0, g1):
                    sg = slice(g * BC, (g + 1) * BC)
                    nc.vector.tensor_tensor(
                        spl[:, sg], p2[:, sg], q2[:, sg], OP.subtract
                    )
            return spl

        def emit_matmuls(rep, it, spl, kt):
            for g in range(G):
                rhs = rhs_pool.tile(
                    [128, O], MM_DT, tag="rhs", name=f"rhs{rep}_{it}_{g}"
                )
                nc.sync.dma_start(rhs[:], coefT[g, it * 128 : (it + 1) * 128, :])
                first = kt == 0
                for m in range(4):
                    lhsT = spl[:, g * BC + m * 128 : g * BC + (m + 1) * 128]
                    for h in range(2):
                        nc.tensor.matmul(
                            psum[m][h][:],
                            lhsT,
                            rhs[:, h * 512 : (h + 1) * 512],
                            start=first,
                            stop=False,
                        )
                kt += 1
            return kt

        def emit_last_matmuls(rep, it, spl, ot):
            """Last i-tile: (m,h)-outer, g-inner so each PSUM bank finishes
            its accumulation early and its drain (copy + DMA piece) overlaps
            the remaining stop-matmuls instead of serializing at the end."""
            rhs = []
            for g in range(G):
                r = rhs_pool.tile([128, O], MM_DT, tag="rhs", name=f"rhs{rep}_{it}_{g}")
                nc.sync.dma_start(r[:], coefT[g, it * 128 : (it + 1) * 128, :])
                rhs.append(r)
            for m in range(4):
                for h in range(2):
                    s = slice(h * 512, (h + 1) * 512)
                    for g in range(G):
                        lhsT = spl[:, g * BC + m * 128 : g * BC + (m + 1) * 128]
                        nc.tensor.matmul(
                            psum[m][h][:],
                            lhsT,
                            rhs[g][:, s],
                            start=False,
                            stop=(g == G - 1),
                        )
                    if (m + h) % 2 == 0:
                        nc.scalar.copy(ot[m][:, s], psum[m][h][:])
                    else:
                        nc.vector.tensor_scalar(
                            ot[m][:, s], psum[m][h][:], 1.0, None, OP.mult
                        )
                    nc.sync.dma_start(y[m * 128 : (m + 1) * 128, s], ot[m][:, s])

        for _rep in range(repeats):
            # software-pipelined emission: basis(it+1) is emitted before the
            # matmuls of it, so ACT/DVE run ahead of the PE.
            kt = 0
            ot = [
                out_pool.tile([128, O], F16, tag=f"ot{m}", name=f"ot{_rep}_{m}")
                for m in range(4)
            ]
            def load_xt(it):
                xt = xt_pool.tile(
                    [128, BC], F32, tag=f"xt{it % 2}", name=f"xt{_rep}_{it}"
                )
                nc.sync.dma_start(xt[:], xT[it * 128 : (it + 1) * 128, :])
                return xt

            # fine plane-groups for the first two tiles so the PE starts
            # early and isn't starved at the tile-0 -> tile-1 transition
            if _rep == 0:
                bounds_by_it = {
                    0: tuple((g, g + 1) for g in range(G)),
                }
                abs_by_it = {}
            else:
                bounds_by_it = {}
                abs_by_it = {}
            xt = load_xt(0)
            xt_next = load_xt(1)
            if _rep == 0:
                emit_warmup(WARM_FILL, first=True)
            spl = emit_basis(
                _rep, 0, xt, bounds_by_it.get(0, ((0, G),)), abs_by_it.get(0, ())
            )
            for it in range(1, IT):
                xt, xt_next = xt_next, (load_xt(it + 1) if it + 1 < IT else None)
                spl_next = emit_basis(
                    _rep, it, xt, bounds_by_it.get(it, ((0, G),)), abs_by_it.get(it, ())
                )
                kt = emit_matmuls(_rep, it - 1, spl, kt)
                if _rep == 0 and it == 1:
                    emit_warmup(WARM_T1)
                spl = spl_next
            emit_last_matmuls(_rep, IT - 1, spl, ot)

    nc.compile()
    return nc


def make_in_maps(x: np.ndarray, coef: np.ndarray):
    """Shard FULL inputs into per-core input maps (host-side prep)."""
    xT = np.ascontiguousarray(x.T)  # [I, B]
    coefT = np.ascontiguousarray(
        (coef.transpose(2, 1, 0)[:G] / 6.0).astype(np.float16)
    )  # [7, I, O] fp16, 1/6 folded in
    return [
        {
            "xT": np.ascontiguousarray(xT[:, c * BC : (c + 1) * BC]),
            "coefT": coefT,
        }
        for c in range(NCORES)
    ]


def kernel(x: np.ndarray, coef: np.ndarray) -> np.ndarray:
    global LAST_RESULT
    x = np.asarray(x, dtype=np.float32)
    coef = np.asarray(coef, dtype=np.float32)
    assert x.shape == (B, I) and coef.shape == (O, I, 8)

    if "nc" not in _cache:
        _cache["nc"] = _build_nc()
    nc = _cache["nc"]

    in_maps = make_in_maps(x, coef)
    res = run_bass_kernel_spmd(nc, in_maps, list(range(NCORES)))
    LAST_RESULT = res
    out = np.concatenate([res.results[c]["y"] for c in range(NCORES)], axis=0)
    return np.ascontiguousarray(out.astype(np.float32))


if __name__ == "__main__":
    rng = np.random.default_rng(0)
    x = rng.standard_normal((B, I), dtype=np.float32)
    coef = rng.standard_normal((O, I, 8), dtype=np.float32) * 0.1
    out = kernel(x, coef)
    print("out", out.shape, out.dtype, float(np.abs(out).max()))
